# revision 37
# baseline (speedup 1.0000x reference)
"""GPT2 causal attention (B=2, T=2048, C=1024, H=16) on 8 TRN2 NeuronCores.

Sharding: core g = (batch b = g//4, head-group hg = g%4 of 4 heads).
Tensor-parallel over heads (column-split W_attn, row-split W_proj) x
data-parallel over batch. Each core computes a full [T, C] partial of the
output projection for its 4 heads; host sums the 4 partials per batch and
adds b_proj. No collectives.

v4 (136969 -> 114718 ns):
  - fp8e4m3 DoubleRow QKV: K=256 per pass at 0.5 PE cycles/row (4x bf16
    throughput). Accuracy preserved by hi+lo error compensation: host
    splits x and 32*W into fp8 hi + fp8 residual; x@W = xh@wh + xh@wl +
    xl@wh (dropped xl@wl is O(2^-8)), i.e. bf16-level error at 6/8 the
    bf16 cycle cost. The 32x weight scale (fp8 subnormal avoidance) is
    folded into the exp scale (scores carry 32*32) and the V
    ones-column (32.0, so the softmax denominator tracks the 32x-scaled
    numerator and the reciprocal-normalize cancels everything exactly).
  - t-banded pipeline: x packed band-major (4 bands of 512 t, 8 c-chunks
    contiguous, hi|lo) so band-0 QKV completes ~7us in and the exp
    stream starts immediately; remaining bands + dc1 weights are filler
    groups paced into the attention slots (qk groups gated before each
    dependent block, V tiles gated before the av that consumes them).
  - globally continuous score pipeline: one score tile per slot streams
    across block boundaries (hp-major order) so Act never drains at a
    block edge; A*V for a tile runs KLAG=12 slots after its score (exp
    long done - the PE never parks on Act), chunk completions run
    recip/normalize immediately but transpose/yT-copy/projection KBLAG=5
    slots later, so no in-order engine queue head-blocks on a
    cross-engine chain.
  - diagonal tri-masks on the otherwise idle Pool engine, consumed last
    in each av (q4 descending) to hide the mask latency; the final
    band's output staging splits Act/DVE since Act is idle post-exp.
"""

import numpy as np
import ml_dtypes

BF16 = ml_dtypes.bfloat16
F8 = ml_dtypes.float8_e4m3fn

B, T, C, H, D = 2, 2048, 1024, 16, 64
HL = 4          # heads per core
DL = HL * D     # 256 local head dims
N_CORES = 8
NT = T // 128   # 16 tk tiles
NJ = T // 512   # 4 tq groups per head pair
SCALE = 1.0 / np.sqrt(D)
WS = 32.0       # host-side weight pre-scale (fp8 subnormal avoidance)

_CACHE = {}


def _build_program():
    import concourse.tile as tile
    from concourse import bacc
    import concourse.mybir as mybir

    f32 = mybir.dt.float32
    f16 = mybir.dt.float16
    bf16 = mybir.dt.bfloat16
    fp8 = mybir.dt.float8e4
    DR = mybir.MatmulPerfMode.DoubleRow
    Exp = mybir.ActivationFunctionType.Exp

    nc = bacc.Bacc("TRN2", target_bir_lowering=False, debug=False)

    # ---- DRAM I/O (host pre-sharded and pre-packed to SBUF layout) ----
    # x: band ts at [ts*8192 + h*4096 + c*512 + t'] (h = hi/lo fp8 split)
    x_d = nc.dram_tensor("xp", [128, 32768], fp8, kind="ExternalInput").ap()
    # per-dc weight packs: [wkh | wkl | wqh | wql], each [128, 1024] c-major
    w0_d = nc.dram_tensor("w0p", [128, 4096], fp8, kind="ExternalInput").ap()
    w1_d = nc.dram_tensor("w1p", [128, 4096], fp8, kind="ExternalInput").ap()
    wv_d = nc.dram_tensor("wvp", [128, 4096], fp8, kind="ExternalInput").ap()
    wp_d = nc.dram_tensor("wpp", [128, 2048], bf16, kind="ExternalInput").ap()
    m32_d = nc.dram_tensor("m32", [128, 260], f32, kind="ExternalInput").ap()
    mbf_d = nc.dram_tensor("mbf", [128, 256], bf16, kind="ExternalInput").ap()
    out_d = nc.dram_tensor("out", [T, C], f16, kind="ExternalOutput").ap()
    _env = __import__("os").environ
    DBG = bool(_env.get("KDBG"))
    KORDER = _env.get("KORDER", "hp")     # pair | hp
    KTRI = _env.get("KTRI", "pool")       # pool | dve
    KPACE = int(_env.get("KPACE", "2"))   # filler pop every KPACE tiles
    KLAG = int(_env.get("KLAG", "12"))    # av lags its score by KLAG slots
    KEXP = int(_env.get("KEXP", "20"))    # et pool depth
    KOUT = int(_env.get("KOUT", "0"))     # 1 = merged per-band out DMA
    KBLAG = int(_env.get("KBLAG", "5"))   # complB/proj age (slots) before run
    KM32 = int(_env.get("KM32", "1"))     # 1 = m32/mbf before x band0
    if DBG:
        qT_dump = nc.dram_tensor("qTd", [128, 2 * T], bf16, kind="ExternalOutput").ap()
        kT_dump = nc.dram_tensor("kTd", [128, 2 * T], bf16, kind="ExternalOutput").ap()
        yT_dump = nc.dram_tensor("yTd", [128, 2 * T], bf16, kind="ExternalOutput").ap()
        V_dump = nc.dram_tensor("Vd", [128, NT * HL * 65], bf16, kind="ExternalOutput").ap()

    with tile.TileContext(nc) as tc:
        with (
            tc.tile_pool(name="const", bufs=1) as cpool,
            tc.tile_pool(name="exp", bufs=KEXP) as epool,
            tc.tile_pool(name="small", bufs=8) as spool,
            tc.tile_pool(name="ostage", bufs=6) as opool,
            tc.tile_pool(name="pssc", bufs=2, space="PSUM") as pool_sc,
            tc.tile_pool(name="psy", bufs=2, space="PSUM") as pool_yps,
            tc.tile_pool(name="pspt", bufs=2, space="PSUM") as pool_pt,
        ):
            # ---- persistent SBUF ----
            # x band-major: band ts, hi/lo h, chunk c at
            # [:, ts*8192 + h*4096 + c*512 + t']
            xb = cpool.tile([128, 32768], fp8, tag="xb")
            w0 = cpool.tile([128, 4096], fp8, tag="w0")     # dc0: kh|kl|qh|ql
            w1 = cpool.tile([128, 4096], fp8, tag="w1")     # dc1: kh|kl|qh|ql
            wv2 = cpool.tile([128, 4096], fp8, tag="wv2")   # vh|vl, c*256
            wp = cpool.tile([128, 2048], bf16, tag="wp")    # dc*1024 + cols
            m32 = cpool.tile([128, 260], f32, tag="m32")    # bq|bk|bvr (x32)
            mbf = cpool.tile([128, 256], bf16, tag="mbf")   # tri|eye
            bq = m32[:, 0:2]
            bk = m32[:, 2:4]
            bvr = m32[:, 4:260]
            tri = mbf[:, 0:128]
            eye = mbf[:, 128:256]
            qT = cpool.tile([128, 2 * T], bf16, tag="qT")   # head h: [64*(h%2):, (h//2)*T + t]
            kT = cpool.tile([128, 2 * T], bf16, tag="kT")
            yT = cpool.tile([128, 2 * T], bf16, tag="yT")   # pair hp at [:, hp*T + t]
            # V_aug bf16, head-major: slot (h,tt) at [:, h*NT*65 + tt*65 : +65],
            # col 64 = 32.0 (so A*V yields 32x softmax row-sums, matching the
            # 32x-scaled y numerator; the reciprocal-normalize cancels both).
            V = cpool.tile([128, HL * NT * 65], bf16, tag="V")

            Vv = V[:, :].rearrange("p (h t e) -> p h t e", h=HL, t=NT)
            nc.vector.memset(Vv[:, :, :, 64:65], float(WS))

            # ---- load inputs; HWDGE descgen + the DMA engine serialize
            # transfers, so few-but-large DMAs in need order. Band 0 split
            # finer so the first Q/K matmuls can chase the arriving pieces.
            nc.sync.dma_start(out=w0[:, 0:2048], in_=w0_d[:, 0:2048])   # k hi|lo
            nc.sync.dma_start(out=m32[:, :], in_=m32_d[:, :])
            nc.sync.dma_start(out=mbf[:, :], in_=mbf_d[:, :])
            for lo in (0, 4096, 2048, 6144):  # hi a01, lo a01, hi a23, lo a23
                nc.sync.dma_start(out=xb[:, lo:lo + 2048],
                                  in_=x_d[:, lo:lo + 2048])
            nc.sync.dma_start(out=w0[:, 2048:4096], in_=w0_d[:, 2048:4096])
            nc.sync.dma_start(out=wv2[:, :], in_=wv_d[:, :])
            nc.sync.dma_start(out=xb[:, 8192:16384], in_=x_d[:, 8192:16384])
            nc.sync.dma_start(out=w1[:, :], in_=w1_d[:, :])
            nc.sync.dma_start(out=xb[:, 16384:24576], in_=x_d[:, 16384:24576])
            nc.sync.dma_start(out=wp[:, :], in_=wp_d[:, :])
            nc.sync.dma_start(out=xb[:, 24576:32768], in_=x_d[:, 24576:32768])

            # ---- QKV band groups: fp8 DoubleRow, hi/lo compensated ----
            def qk_pair_mms(ps, wdc, wo, ts, a, start, stop):
                # one K=256 pair (c = 2a, 2a+1): xh@wh + xh@wl + xl@wh
                # wdc = w0/w1 pack, wo = 0 for K, 2048 for Q
                xo = ts * 8192 + 1024 * a
                lh = wdc[:, wo + 256 * a: wo + 256 * a + 256].rearrange(
                    "p (k m) -> p k m", k=2)
                ll = wdc[:, wo + 1024 + 256 * a: wo + 1024 + 256 * a + 256].rearrange(
                    "p (k m) -> p k m", k=2)
                rh = xb[:, xo:xo + 1024].rearrange("p (k t) -> p k t", k=2)
                rl = xb[:, 4096 + xo:4096 + xo + 1024].rearrange(
                    "p (k t) -> p k t", k=2)
                nc.tensor.matmul(ps[:, :], lh, rh, start=start, stop=False,
                                 perf_mode=DR)
                nc.tensor.matmul(ps[:, :], ll, rh, start=False, stop=False,
                                 perf_mode=DR)
                nc.tensor.matmul(ps[:, :], lh, rl, start=False, stop=stop,
                                 perf_mode=DR)

            def emit_qk_band(wdc, wo, b_sb, dst, dc, ts):
                ps = pool_pt.tile([128, 512], f32, tag="pt",
                                  name=f"qk{dc}_{ts}")
                for a in range(4):
                    qk_pair_mms(ps, wdc, wo, ts, a, a == 0, a == 3)
                nc.vector.tensor_scalar_add(
                    dst[:, dc * T + ts * 512: dc * T + (ts + 1) * 512],
                    ps[:, :], b_sb[:, dc:dc + 1],
                )

            def emit_qk_band0():
                # band 0 Q/K interleaved per pair (chases the arriving DMA)
                psk = pool_pt.tile([128, 512], f32, tag="pt", name="qk0_k0")
                psq = pool_pt.tile([128, 512], f32, tag="pt", name="qk0_q0")
                for a in range(4):
                    qk_pair_mms(psk, w0, 0, 0, a, a == 0, a == 3)
                    qk_pair_mms(psq, w0, 2048, 0, a, a == 0, a == 3)
                nc.vector.tensor_scalar_add(kT[:, 0:512], psk[:, :], bk[:, 0:1])
                nc.vector.tensor_scalar_add(qT[:, 0:512], psq[:, :], bq[:, 0:1])

            def emit_v_tile(tt):
                ts, k = divmod(tt, 4)
                ps = pool_pt.tile([128, DL], f32, tag="pt", name=f"vps{tt}")
                xhb = xb[:, ts * 8192:ts * 8192 + 4096].rearrange(
                    "p (c t) -> p c t", c=8)
                xlb = xb[:, ts * 8192 + 4096:ts * 8192 + 8192].rearrange(
                    "p (c t) -> p c t", c=8)
                for a in range(4):
                    lh = xhb[:, 2 * a:2 * a + 2, k * 128:(k + 1) * 128]
                    ll = xlb[:, 2 * a:2 * a + 2, k * 128:(k + 1) * 128]
                    rh = wv2[:, 512 * a:512 * a + 512].rearrange(
                        "p (k e) -> p k e", k=2)
                    rl = wv2[:, 2048 + 512 * a:2048 + 512 * a + 512].rearrange(
                        "p (k e) -> p k e", k=2)
                    nc.tensor.matmul(ps[:, :], lh, rh, start=(a == 0),
                                     stop=False, perf_mode=DR)
                    nc.tensor.matmul(ps[:, :], ll, rh, start=False,
                                     stop=False, perf_mode=DR)
                    nc.tensor.matmul(ps[:, :], lh, rl, start=False,
                                     stop=(a == 3), perf_mode=DR)
                nc.vector.tensor_add(
                    Vv[:, :, tt, 0:64],
                    ps[:, :].rearrange("p (h e) -> p h e", h=HL),
                    bvr[:, :].rearrange("p (h e) -> p h e", h=HL),
                )

            emit_qk_band0()
            for tt in range(4):
                emit_v_tile(tt)

            # filler groups, in dependency order for interleaved blocks:
            # dc0 band ts + V band ts must precede block (0,ts); dc1 band ts
            # must precede block (1,ts).
            fillers = []
            cum = {}       # groups required before a block's first score
            vpos = {}      # filler count required before av consumes V tile
            for tt in range(4):
                vpos[tt] = 0
            for ts in range(4):
                if ts > 0:
                    fillers.append(
                        lambda ts=ts: emit_qk_band(w0, 0, bk, kT, 0, ts))
                    fillers.append(
                        lambda ts=ts: emit_qk_band(w0, 2048, bq, qT, 0, ts))
                    cum[(0, ts)] = len(fillers)
                fillers.append(
                    lambda ts=ts: emit_qk_band(w1, 0, bk, kT, 1, ts))
                fillers.append(
                    lambda ts=ts: emit_qk_band(w1, 2048, bq, qT, 1, ts))
                cum[(1, ts)] = len(fillers)
                if ts > 0:
                    for k in range(4):
                        fillers.append(lambda tt=4 * ts + k: emit_v_tile(tt))
                        vpos[4 * ts + k] = len(fillers)
            fillers.reverse()  # pop() from the front
            n_popped = [0]

            def pop_filler():
                if fillers:
                    fillers.pop()()
                    n_popped[0] += 1

            # ---- attention ----
            def emit_proj(tt, tail=False):
                # out[tt band, :] = sum_dc yT[dc, tt]^T @ wp[dc]; fp16 partial out
                ot = opool.tile([128, 1024], f16, tag="ot", name=f"ot{tt}")
                for cc in range(2):
                    pp = pool_pt.tile([128, 512], f32, tag="pt", name=f"pp{tt}_{cc}")
                    for dc in range(2):
                        nc.tensor.matmul(
                            pp[:, :],
                            yT[:, dc * T + tt * 128: dc * T + (tt + 1) * 128],
                            wp[:, dc * C + cc * 512: dc * C + (cc + 1) * 512],
                            start=(dc == 0), stop=(dc == 1),
                        )
                    osl = ot[:, cc * 512:(cc + 1) * 512]
                    if tail and cc == 0:  # Act is idle after the last exps
                        nc.scalar.copy(osl, pp[:, :])
                    else:
                        nc.vector.tensor_copy(osl, pp[:, :])
                    if not KOUT:
                        nc.sync.dma_start(
                            out=out_d[tt * 128:(tt + 1) * 128,
                                      cc * 512:(cc + 1) * 512],
                            in_=ot[:, cc * 512:(cc + 1) * 512],
                        )
                if KOUT:
                    nc.sync.dma_start(
                        out=out_d[tt * 128:(tt + 1) * 128, :],
                        in_=ot[:, :],
                    )

            def emit_complA(hp, j, q4, ytiles):
                # q-chunk jj = 4j+q4 finished accumulating: normalize both
                # heads' [128 q, 64] by the denominators (col 64 of each
                # slot) into a staged yp tile. Frees the yps slot pair.
                jj = 4 * j + q4
                yt = ytiles[q4 // 2]
                base = 132 * (q4 % 2)
                dn = yt[:, :].rearrange("p (s e) -> p s e", s=4)[
                    :, 2 * (q4 % 2):2 * (q4 % 2) + 2, 64]
                rc = spool.tile([128, 2], f32, tag="rc", name=f"rc{hp}_{jj}")
                nc.vector.reciprocal(rc[:, :], dn)
                yp = spool.tile([128, 128], bf16, tag="yp", name=f"yp{hp}_{jj}")
                for half in range(2):
                    nc.vector.tensor_scalar_mul(
                        yp[:, half * 64:(half + 1) * 64],
                        yt[:, base + half * 66: base + half * 66 + 64],
                        rc[:, half:half + 1],
                    )
                return yp

            def emit_complB_T(hp, jj, yp):
                # transpose the staged [q, d] chunk into yT's [d, q] layout
                tp = pool_pt.tile([128, 128], bf16, tag="pt", name=f"tp{hp}_{jj}")
                nc.tensor.transpose(tp[:, :], yp[:, :], eye[:, :])
                nc.vector.tensor_copy(yT[:, hp * T + jj * 128: hp * T + (jj + 1) * 128],
                                      tp[:, :])

            def attn_block(hp, j):
                fb = hp * T
                ni = 4 * j + 4
                # yps slot (q4, half) = 2*q4+half: slots 0-3 in ya, 4-7 in yb;
                # 66 cols each (65 used: col 64 = softmax denominator).
                ytiles = [
                    pool_yps.tile([128, 264], f32, tag="yps", name=f"y{hp}_{j}_{m}")
                    for m in range(2)
                ]
                ets = [None] * ni

                def emit_score(i):
                    d0 = max(128 * (i - 4 * j), 0)
                    sc = pool_sc.tile([128, 1024], f32, tag="sc",
                                      name=f"sc{hp}_{j}_{i}")
                    for half in range(2):
                        po = 64 * half
                        nc.tensor.matmul(
                            sc[:, half * 512 + d0:(half + 1) * 512],
                            kT[po:po + 64, fb + i * 128: fb + (i + 1) * 128],
                            qT[po:po + 64, fb + j * 512 + d0: fb + (j + 1) * 512],
                            start=True, stop=True,
                        )
                    et = epool.tile([128, 1024], bf16, tag="exp",
                                    name=f"et{hp}_{j}_{i}")
                    et2 = et[:, :].rearrange("p (g q) -> p g q", g=2)
                    sc2 = sc[:, :].rearrange("p (g q) -> p g q", g=2)
                    nc.scalar.activation(
                        et2[:, :, d0:512], sc2[:, :, d0:512], Exp,
                        scale=float(SCALE / (WS * WS)),
                    )
                    if i >= 4 * j:  # diagonal chunk: causal mask (post-exp)
                        teng = nc.gpsimd if KTRI == "pool" else nc.vector
                        for half in range(2):
                            sl = slice(half * 512 + d0, half * 512 + d0 + 128)
                            teng.tensor_mul(et[:, sl], et[:, sl], tri[:, :])
                    ets[i] = et

                def emit_av(i):
                    # PSUM start_tensor_calc marks the whole 2KB bank pending-
                    # zero, so: ONE start per yps bank (its first matmul); the
                    # other slots' first writes land on pending-zero bytes and
                    # overwrite; ONE stop on the bank's last matmul. q4 runs
                    # descending so the tri-masked diagonal chunk (q4 = i-4j)
                    # is consumed last, hiding the Pool mask latency.
                    et = ets[i]
                    for half in range(2):
                        h = 2 * hp + half
                        for q4 in range(3, -1, -1):
                            if 4 * j + q4 < i:
                                continue
                            s = 2 * q4 + half
                            yt = ytiles[s // 4]
                            off = (s % 4) * 66
                            bank_start = (i == 0 and half == 0
                                          and q4 % 2 == 1)
                            bank_stop = (half == 1 and q4 % 2 == 1
                                         and i == 4 * j + q4)
                            nc.tensor.matmul(
                                yt[:, off:off + 65],
                                et[:, half * 512 + q4 * 128: half * 512 + (q4 + 1) * 128],
                                Vv[:, h, i, :],
                                start=bank_start, stop=bank_stop,
                                skip_group_check=True,
                            )

                # yield one closure bundle per score tile; the global driver
                # interleaves score slots across block boundaries so the Act
                # exp stream never drains at a boundary.
                def make_av(i):
                    def run():
                        if hp == 0 and i >= 4 * j:
                            # V tile i filler must be emitted before av(i)
                            while n_popped[0] < vpos.get(i, 0):
                                pop_filler()
                        emit_av(i)
                        qa = i - 4 * j
                        if qa >= 0:
                            yp = emit_complA(hp, j, qa, ytiles)
                            bq_.append((slot[0] + KBLAG,
                                        lambda: emit_complB_T(hp, 4 * j + qa, yp)))
                            if hp == 1:
                                tail = (j == NJ - 1)
                                pq_.append((slot[0] + KBLAG + 1,
                                            lambda: emit_proj(4 * j + qa, tail)))
                    return run

                pace = KPACE if KPACE != 9 else (1 if j >= 2 else 3)
                for i in range(ni):
                    yield emit_score, i, make_av(i), pace

            if KORDER == "pair":
                order = [(hp, j) for j in range(NJ) for hp in range(2)]
            elif KORDER == "hprev":
                order = ([(0, j) for j in range(NJ)]
                         + [(1, j) for j in range(NJ - 1, -1, -1)])
            else:
                order = [(hp, j) for hp in range(2) for j in range(NJ)]
            req_so_far = 0
            reqs = []
            for hp, j in order:
                req_so_far = max(req_so_far, cum.get((hp, j), 0))
                reqs.append(req_so_far)

            # ---- global slot driver: one score per slot, continuous ----
            bq_ = []        # pending complB (transpose+copy), 1 per slot
            pq_ = []        # pending proj, 1 per slot
            avq_ = []       # pending av closures, run KLAG slots late
            slot = [0]

            def run_slot(score_fn=None, si=None, av_fn=None, pace=None):
                if score_fn is not None:
                    score_fn(si)
                if av_fn is not None:
                    avq_.append(av_fn)
                if bq_ and (bq_[0][0] <= slot[0] or score_fn is None):
                    bq_.pop(0)[1]()
                if slot[0] % (pace or 2) == 0:
                    pop_filler()
                if pq_ and (pq_[0][0] <= slot[0] or score_fn is None):
                    pq_.pop(0)[1]()
                while len(avq_) > (KLAG - 1 if score_fn is not None else 0):
                    avq_.pop(0)()
                slot[0] += 1

            for (hp, j), req in zip(order, reqs):
                while n_popped[0] < req:
                    pop_filler()
                for score_fn, si, av_fn, pace in attn_block(hp, j):
                    run_slot(score_fn, si, av_fn, pace)
            # flush: trailing avs, completions, projections, leftover fillers
            for _ in range(2 + KLAG):
                run_slot()
            while fillers:
                pop_filler()

            if DBG:
                nc.sync.dma_start(out=qT_dump[:, :], in_=qT[:, :])
                nc.sync.dma_start(out=kT_dump[:, :], in_=kT[:, :])
                nc.sync.dma_start(out=yT_dump[:, :], in_=yT[:, :])
                nc.sync.dma_start(out=V_dump[:, :], in_=V[:, :])

    nc.compile()
    return nc


def get_program():
    if "nc" not in _CACHE:
        _CACHE["nc"] = _build_program()
    return _CACHE["nc"]


def _pack_cmajor(a):
    """[C_rows, N] -> [128, (C_rows/128)*N] with chunk c at [:, c*N:(c+1)*N]."""
    rows, n = a.shape
    return np.ascontiguousarray(
        a.reshape(rows // 128, 128, n).transpose(1, 0, 2).reshape(128, -1))


def _pack_banded(a):
    """x[b].T [1024, 2048] -> [128, 16384], col = ts*4096 + c*512 + t'."""
    return np.ascontiguousarray(
        a.reshape(8, 128, 4, 512).transpose(1, 2, 0, 3).reshape(128, 16384))


def _split_fp8(a):
    """a (f32) -> (hi, lo) fp8e4m3 with hi + lo ~= a (error ~2^-8 rel)."""
    hi = a.astype(F8)
    lo = (a - hi.astype(np.float32)).astype(F8)
    return hi, lo


def make_in_maps(x, W_attn, b_attn, W_proj):
    """Host-side sharding: per-core input dict."""
    x = np.asarray(x, np.float32)
    W_attn = np.asarray(W_attn, np.float32)
    b_attn = np.asarray(b_attn, np.float32)
    W_proj = np.asarray(W_proj, np.float32)

    tk = np.arange(128)[:, None]
    tq = np.arange(128)[None, :]
    tri = (tq >= tk).astype(BF16)
    eye = np.eye(128, dtype=BF16)
    mbf = np.ascontiguousarray(np.concatenate([tri, eye], axis=1))

    x_b = []
    for b in range(B):
        hi, lo = _split_fp8(x[b].T)
        hi, lo = _pack_banded(hi), _pack_banded(lo)
        # interleave bands: [ts*8192 + h*4096 + ...]
        xi = np.empty((128, 32768), dtype=F8)
        for ts in range(4):
            xi[:, ts * 8192:ts * 8192 + 4096] = hi[:, ts * 4096:(ts + 1) * 4096]
            xi[:, ts * 8192 + 4096:(ts + 1) * 8192] = lo[:, ts * 4096:(ts + 1) * 4096]
        x_b.append(np.ascontiguousarray(xi))

    in_maps = []
    for g in range(N_CORES):
        b, hg = divmod(g, 4)
        cs = slice(hg * DL, (hg + 1) * DL)
        wqh, wql = _split_fp8(WS * W_attn[:, 0 * C:1 * C][:, cs])
        wkh, wkl = _split_fp8(WS * W_attn[:, 1 * C:2 * C][:, cs])
        wvh, wvl = _split_fp8(WS * W_attn[:, 2 * C:3 * C][:, cs])
        # per-dc packs [kh | kl | qh | ql], each part [128, 1024] c-major
        def _wpack(dc):
            sl = slice(dc * 128, (dc + 1) * 128)
            return np.ascontiguousarray(np.concatenate(
                [_pack_cmajor(wkh[:, sl]), _pack_cmajor(wkl[:, sl]),
                 _pack_cmajor(wqh[:, sl]), _pack_cmajor(wql[:, sl])], axis=1))
        wp = _pack_cmajor(W_proj[cs, :].astype(BF16))
        bq = np.ascontiguousarray(WS * b_attn[0 * C:1 * C][cs].reshape(2, 128).T)
        bk = np.ascontiguousarray(WS * b_attn[1 * C:2 * C][cs].reshape(2, 128).T)
        bvr = np.tile(WS * b_attn[2 * C:3 * C][cs][None, :], (128, 1))
        m32 = np.ascontiguousarray(
            np.concatenate([bq, bk, bvr], axis=1).astype(np.float32))
        in_maps.append({
            "xp": x_b[b],
            "w0p": _wpack(0), "w1p": _wpack(1),
            "wvp": np.ascontiguousarray(np.concatenate(
                [_pack_cmajor(wvh), _pack_cmajor(wvl)], axis=1)),
            "wpp": wp, "m32": m32, "mbf": mbf,
        })
    return in_maps


def assemble_output(results, b_proj):
    """results: per-core dicts with 'out' [T, C] fp16 partials."""
    b_proj = np.asarray(b_proj, np.float32)
    out = np.zeros((B, T, C), np.float32)
    for g in range(N_CORES):
        out[g // 4] += np.asarray(results[g]["out"], np.float32)
    out += b_proj[None, None, :]
    return out


def kernel(x, W_attn, b_attn, W_proj, b_proj):
    from concourse.bass_utils import run_bass_kernel_spmd

    nc = get_program()
    in_maps = make_in_maps(x, W_attn, b_attn, W_proj)
    res = run_bass_kernel_spmd(nc, in_maps, list(range(N_CORES)))
    return assemble_output(res.results, b_proj)


# revision 41
# speedup vs baseline: 1.0090x; 1.0090x over previous
"""GPT2 causal attention (B=2, T=2048, C=1024, H=16) on 8 TRN2 NeuronCores.

Sharding: core g = (batch b = g//4, head-group hg = g%4 of 4 heads).
Tensor-parallel over heads (column-split W_attn, row-split W_proj) x
data-parallel over batch. Each core computes a full [T, C] partial of the
output projection for its 4 heads; host sums the 4 partials per batch and
adds b_proj. No collectives.

v4 (136969 -> 114718 ns):
  - fp8e4m3 DoubleRow QKV: K=256 per pass at 0.5 PE cycles/row (4x bf16
    throughput). Accuracy preserved by hi+lo error compensation: host
    splits x and 32*W into fp8 hi + fp8 residual; x@W = xh@wh + xh@wl +
    xl@wh (dropped xl@wl is O(2^-8)), i.e. bf16-level error at 6/8 the
    bf16 cycle cost. The 32x weight scale (fp8 subnormal avoidance) is
    folded into the exp scale (scores carry 32*32) and the V
    ones-column (32.0, so the softmax denominator tracks the 32x-scaled
    numerator and the reciprocal-normalize cancels everything exactly).
  - t-banded pipeline: x packed band-major (4 bands of 512 t, 8 c-chunks
    contiguous, hi|lo) so band-0 QKV completes ~7us in and the exp
    stream starts immediately; remaining bands + dc1 weights are filler
    groups paced into the attention slots (qk groups gated before each
    dependent block, V tiles gated before the av that consumes them).
  - globally continuous score pipeline: one score tile per slot streams
    across block boundaries (hp-major order) so Act never drains at a
    block edge; A*V for a tile runs KLAG=12 slots after its score (exp
    long done - the PE never parks on Act), chunk completions run
    recip/normalize immediately but transpose/yT-copy/projection KBLAG=5
    slots later, so no in-order engine queue head-blocks on a
    cross-engine chain.
  - diagonal tri-masks on the otherwise idle Pool engine, consumed last
    in each av (q4 descending) to hide the mask latency; the final
    band's output staging splits Act/DVE since Act is idle post-exp.
"""

import numpy as np
import ml_dtypes

BF16 = ml_dtypes.bfloat16
F8 = ml_dtypes.float8_e4m3fn

B, T, C, H, D = 2, 2048, 1024, 16, 64
HL = 4          # heads per core
DL = HL * D     # 256 local head dims
N_CORES = 8
NT = T // 128   # 16 tk tiles
NJ = T // 512   # 4 tq groups per head pair
SCALE = 1.0 / np.sqrt(D)
WS = 32.0       # host-side weight pre-scale (fp8 subnormal avoidance)

_CACHE = {}


def _build_program():
    import concourse.tile as tile
    from concourse import bacc
    import concourse.mybir as mybir

    f32 = mybir.dt.float32
    f16 = mybir.dt.float16
    bf16 = mybir.dt.bfloat16
    fp8 = mybir.dt.float8e4
    DR = mybir.MatmulPerfMode.DoubleRow
    Exp = mybir.ActivationFunctionType.Exp

    nc = bacc.Bacc("TRN2", target_bir_lowering=False, debug=False)

    # ---- DRAM I/O (host pre-sharded and pre-packed to SBUF layout) ----
    # x: band ts at [ts*8192 + h*4096 + c*512 + t'] (h = hi/lo fp8 split)
    x_d = nc.dram_tensor("xp", [128, 32768], fp8, kind="ExternalInput").ap()
    # per-dc weight packs: [wkh | wkl | wqh | wql], each [128, 1024] c-major
    w0_d = nc.dram_tensor("w0p", [128, 4096], fp8, kind="ExternalInput").ap()
    w1_d = nc.dram_tensor("w1p", [128, 4096], fp8, kind="ExternalInput").ap()
    wv_d = nc.dram_tensor("wvp", [128, 4096], fp8, kind="ExternalInput").ap()
    wp_d = nc.dram_tensor("wpp", [128, 2048], bf16, kind="ExternalInput").ap()
    m32_d = nc.dram_tensor("m32", [128, 260], f32, kind="ExternalInput").ap()
    mbf_d = nc.dram_tensor("mbf", [128, 256], bf16, kind="ExternalInput").ap()
    out_d = nc.dram_tensor("out", [T, C], f16, kind="ExternalOutput").ap()
    _env = __import__("os").environ
    DBG = bool(_env.get("KDBG"))
    KORDER = _env.get("KORDER", "hp")     # pair | hp
    KTRI = _env.get("KTRI", "pool")       # pool | dve
    KPACE = int(_env.get("KPACE", "2"))   # filler pop every KPACE tiles
    KLAG = int(_env.get("KLAG", "12"))    # av lags its score by KLAG slots
    KEXP = int(_env.get("KEXP", "20"))    # et pool depth
    KOUT = int(_env.get("KOUT", "0"))     # 1 = merged per-band out DMA
    KBLAG = int(_env.get("KBLAG", "4"))   # complB/proj age (slots) before run
    KM32 = int(_env.get("KM32", "1"))     # 1 = m32/mbf before x band0
    KPRI = int(_env.get("KPRI", "0"))     # score priority hoist (-1=off, 0=max)
    KPRIB = int(_env.get("KPRIB", "-1"))  # qk-band/evict priority hoist
    KPRA = int(_env.get("KPRA", "0"))     # av priority delay (>0 = later)
    KPRC = int(_env.get("KPRC", "-1"))    # complA priority hoist
    if DBG:
        qT_dump = nc.dram_tensor("qTd", [128, 2 * T], bf16, kind="ExternalOutput").ap()
        kT_dump = nc.dram_tensor("kTd", [128, 2 * T], bf16, kind="ExternalOutput").ap()
        yT_dump = nc.dram_tensor("yTd", [128, 2 * T], bf16, kind="ExternalOutput").ap()
        V_dump = nc.dram_tensor("Vd", [128, NT * HL * 65], bf16, kind="ExternalOutput").ap()

    with tile.TileContext(nc) as tc:
        with (
            tc.tile_pool(name="const", bufs=1) as cpool,
            tc.tile_pool(name="exp", bufs=KEXP) as epool,
            tc.tile_pool(name="small", bufs=8) as spool,
            tc.tile_pool(name="ostage", bufs=6) as opool,
            tc.tile_pool(name="pssc", bufs=2, space="PSUM") as pool_sc,
            tc.tile_pool(name="psy", bufs=2, space="PSUM") as pool_yps,
            tc.tile_pool(name="pspt", bufs=2, space="PSUM") as pool_pt,
        ):
            # ---- persistent SBUF ----
            # x band-major: band ts, hi/lo h, chunk c at
            # [:, ts*8192 + h*4096 + c*512 + t']
            xb = cpool.tile([128, 32768], fp8, tag="xb")
            w0 = cpool.tile([128, 4096], fp8, tag="w0")     # dc0: kh|kl|qh|ql
            w1 = cpool.tile([128, 4096], fp8, tag="w1")     # dc1: kh|kl|qh|ql
            wv2 = cpool.tile([128, 4096], fp8, tag="wv2")   # vh|vl, c*256
            wp = cpool.tile([128, 2048], bf16, tag="wp")    # dc*1024 + cols
            m32 = cpool.tile([128, 260], f32, tag="m32")    # bq|bk|bvr (x32)
            mbf = cpool.tile([128, 256], bf16, tag="mbf")   # tri|eye
            bq = m32[:, 0:2]
            bk = m32[:, 2:4]
            bvr = m32[:, 4:260]
            tri = mbf[:, 0:128]
            eye = mbf[:, 128:256]
            qT = cpool.tile([128, 2 * T], bf16, tag="qT")   # head h: [64*(h%2):, (h//2)*T + t]
            kT = cpool.tile([128, 2 * T], bf16, tag="kT")
            yT = cpool.tile([128, 2 * T], bf16, tag="yT")   # pair hp at [:, hp*T + t]
            # V_aug bf16, head-major: slot (h,tt) at [:, h*NT*65 + tt*65 : +65],
            # col 64 = 32.0 (so A*V yields 32x softmax row-sums, matching the
            # 32x-scaled y numerator; the reciprocal-normalize cancels both).
            V = cpool.tile([128, HL * NT * 65], bf16, tag="V")

            Vv = V[:, :].rearrange("p (h t e) -> p h t e", h=HL, t=NT)
            nc.vector.memset(Vv[:, :, :, 64:65], float(WS))

            # ---- load inputs; HWDGE descgen + the DMA engine serialize
            # transfers, so few-but-large DMAs in need order. Band 0 split
            # finer so the first Q/K matmuls can chase the arriving pieces.
            nc.sync.dma_start(out=w0[:, 0:2048], in_=w0_d[:, 0:2048])   # k hi|lo
            nc.sync.dma_start(out=m32[:, :], in_=m32_d[:, :])
            nc.sync.dma_start(out=mbf[:, :], in_=mbf_d[:, :])
            for lo in (0, 4096, 2048, 6144):  # hi a01, lo a01, hi a23, lo a23
                nc.sync.dma_start(out=xb[:, lo:lo + 2048],
                                  in_=x_d[:, lo:lo + 2048])
            nc.sync.dma_start(out=w0[:, 2048:4096], in_=w0_d[:, 2048:4096])
            nc.sync.dma_start(out=wv2[:, :], in_=wv_d[:, :])
            nc.sync.dma_start(out=xb[:, 8192:16384], in_=x_d[:, 8192:16384])
            nc.sync.dma_start(out=w1[:, :], in_=w1_d[:, :])
            nc.sync.dma_start(out=xb[:, 16384:24576], in_=x_d[:, 16384:24576])
            nc.sync.dma_start(out=wp[:, :], in_=wp_d[:, :])
            nc.sync.dma_start(out=xb[:, 24576:32768], in_=x_d[:, 24576:32768])

            # ---- QKV band groups: fp8 DoubleRow, hi/lo compensated ----
            def qk_pair_mms(ps, wdc, wo, ts, a, start, stop):
                # one K=256 pair (c = 2a, 2a+1): xh@wh + xh@wl + xl@wh
                # wdc = w0/w1 pack, wo = 0 for K, 2048 for Q
                xo = ts * 8192 + 1024 * a
                lh = wdc[:, wo + 256 * a: wo + 256 * a + 256].rearrange(
                    "p (k m) -> p k m", k=2)
                ll = wdc[:, wo + 1024 + 256 * a: wo + 1024 + 256 * a + 256].rearrange(
                    "p (k m) -> p k m", k=2)
                rh = xb[:, xo:xo + 1024].rearrange("p (k t) -> p k t", k=2)
                rl = xb[:, 4096 + xo:4096 + xo + 1024].rearrange(
                    "p (k t) -> p k t", k=2)
                nc.tensor.matmul(ps[:, :], lh, rh, start=start, stop=False,
                                 perf_mode=DR)
                nc.tensor.matmul(ps[:, :], ll, rh, start=False, stop=False,
                                 perf_mode=DR)
                nc.tensor.matmul(ps[:, :], lh, rl, start=False, stop=stop,
                                 perf_mode=DR)

            def emit_qk_band(wdc, wo, b_sb, dst, dc, ts):
                import contextlib
                hoist = (tc.high_priority(None if KPRIB == 0 else KPRIB)
                         if KPRIB >= 0 else contextlib.nullcontext())
                ps = pool_pt.tile([128, 512], f32, tag="pt",
                                  name=f"qk{dc}_{ts}")
                for a in range(4):
                    qk_pair_mms(ps, wdc, wo, ts, a, a == 0, a == 3)
                with hoist:
                    nc.vector.tensor_scalar_add(
                        dst[:, dc * T + ts * 512: dc * T + (ts + 1) * 512],
                        ps[:, :], b_sb[:, dc:dc + 1],
                    )

            def emit_qk_band0():
                # band 0 Q/K interleaved per pair (chases the arriving DMA)
                psk = pool_pt.tile([128, 512], f32, tag="pt", name="qk0_k0")
                psq = pool_pt.tile([128, 512], f32, tag="pt", name="qk0_q0")
                for a in range(4):
                    qk_pair_mms(psk, w0, 0, 0, a, a == 0, a == 3)
                    qk_pair_mms(psq, w0, 2048, 0, a, a == 0, a == 3)
                nc.vector.tensor_scalar_add(kT[:, 0:512], psk[:, :], bk[:, 0:1])
                nc.vector.tensor_scalar_add(qT[:, 0:512], psq[:, :], bq[:, 0:1])

            def emit_v_tile(tt):
                ts, k = divmod(tt, 4)
                ps = pool_pt.tile([128, DL], f32, tag="pt", name=f"vps{tt}")
                xhb = xb[:, ts * 8192:ts * 8192 + 4096].rearrange(
                    "p (c t) -> p c t", c=8)
                xlb = xb[:, ts * 8192 + 4096:ts * 8192 + 8192].rearrange(
                    "p (c t) -> p c t", c=8)
                for a in range(4):
                    lh = xhb[:, 2 * a:2 * a + 2, k * 128:(k + 1) * 128]
                    ll = xlb[:, 2 * a:2 * a + 2, k * 128:(k + 1) * 128]
                    rh = wv2[:, 512 * a:512 * a + 512].rearrange(
                        "p (k e) -> p k e", k=2)
                    rl = wv2[:, 2048 + 512 * a:2048 + 512 * a + 512].rearrange(
                        "p (k e) -> p k e", k=2)
                    nc.tensor.matmul(ps[:, :], lh, rh, start=(a == 0),
                                     stop=False, perf_mode=DR)
                    nc.tensor.matmul(ps[:, :], ll, rh, start=False,
                                     stop=False, perf_mode=DR)
                    nc.tensor.matmul(ps[:, :], lh, rl, start=False,
                                     stop=(a == 3), perf_mode=DR)
                nc.vector.tensor_add(
                    Vv[:, :, tt, 0:64],
                    ps[:, :].rearrange("p (h e) -> p h e", h=HL),
                    bvr[:, :].rearrange("p (h e) -> p h e", h=HL),
                )

            emit_qk_band0()
            for tt in range(4):
                emit_v_tile(tt)

            # filler groups, in dependency order for interleaved blocks:
            # dc0 band ts + V band ts must precede block (0,ts); dc1 band ts
            # must precede block (1,ts).
            fillers = []
            cum = {}       # groups required before a block's first score
            vpos = {}      # filler count required before av consumes V tile
            for tt in range(4):
                vpos[tt] = 0
            for ts in range(4):
                if ts > 0:
                    fillers.append(
                        lambda ts=ts: emit_qk_band(w0, 0, bk, kT, 0, ts))
                    fillers.append(
                        lambda ts=ts: emit_qk_band(w0, 2048, bq, qT, 0, ts))
                    cum[(0, ts)] = len(fillers)
                fillers.append(
                    lambda ts=ts: emit_qk_band(w1, 0, bk, kT, 1, ts))
                fillers.append(
                    lambda ts=ts: emit_qk_band(w1, 2048, bq, qT, 1, ts))
                cum[(1, ts)] = len(fillers)
                if ts > 0:
                    for k in range(4):
                        fillers.append(lambda tt=4 * ts + k: emit_v_tile(tt))
                        vpos[4 * ts + k] = len(fillers)
            fillers.reverse()  # pop() from the front
            n_popped = [0]

            def pop_filler():
                if fillers:
                    fillers.pop()()
                    n_popped[0] += 1

            # ---- attention ----
            def emit_proj(tt, tail=False):
                # out[tt band, :] = sum_dc yT[dc, tt]^T @ wp[dc]; fp16 partial out
                ot = opool.tile([128, 1024], f16, tag="ot", name=f"ot{tt}")
                for cc in range(2):
                    pp = pool_pt.tile([128, 512], f32, tag="pt", name=f"pp{tt}_{cc}")
                    for dc in range(2):
                        nc.tensor.matmul(
                            pp[:, :],
                            yT[:, dc * T + tt * 128: dc * T + (tt + 1) * 128],
                            wp[:, dc * C + cc * 512: dc * C + (cc + 1) * 512],
                            start=(dc == 0), stop=(dc == 1),
                        )
                    osl = ot[:, cc * 512:(cc + 1) * 512]
                    if tail and cc == 0:  # Act is idle after the last exps
                        nc.scalar.copy(osl, pp[:, :])
                    else:
                        nc.vector.tensor_copy(osl, pp[:, :])
                    if not KOUT:
                        nc.sync.dma_start(
                            out=out_d[tt * 128:(tt + 1) * 128,
                                      cc * 512:(cc + 1) * 512],
                            in_=ot[:, cc * 512:(cc + 1) * 512],
                        )
                if KOUT:
                    nc.sync.dma_start(
                        out=out_d[tt * 128:(tt + 1) * 128, :],
                        in_=ot[:, :],
                    )

            def emit_complA(hp, j, q4, ytiles):
                import contextlib
                hoist = (tc.high_priority(None if KPRC == 0 else KPRC)
                         if KPRC >= 0 else contextlib.nullcontext())
                with hoist:
                    return _emit_complA(hp, j, q4, ytiles)

            def _emit_complA(hp, j, q4, ytiles):
                # q-chunk jj = 4j+q4 finished accumulating: normalize both
                # heads' [128 q, 64] by the denominators (col 64 of each
                # slot) into a staged yp tile. Frees the yps slot pair.
                jj = 4 * j + q4
                yt = ytiles[q4 // 2]
                base = 132 * (q4 % 2)
                dn = yt[:, :].rearrange("p (s e) -> p s e", s=4)[
                    :, 2 * (q4 % 2):2 * (q4 % 2) + 2, 64]
                rc = spool.tile([128, 2], f32, tag="rc", name=f"rc{hp}_{jj}")
                nc.vector.reciprocal(rc[:, :], dn)
                yp = spool.tile([128, 128], bf16, tag="yp", name=f"yp{hp}_{jj}")
                for half in range(2):
                    nc.vector.tensor_scalar_mul(
                        yp[:, half * 64:(half + 1) * 64],
                        yt[:, base + half * 66: base + half * 66 + 64],
                        rc[:, half:half + 1],
                    )
                return yp

            def emit_complB_T(hp, jj, yp):
                # transpose the staged [q, d] chunk into yT's [d, q] layout
                tp = pool_pt.tile([128, 128], bf16, tag="pt", name=f"tp{hp}_{jj}")
                nc.tensor.transpose(tp[:, :], yp[:, :], eye[:, :])
                nc.vector.tensor_copy(yT[:, hp * T + jj * 128: hp * T + (jj + 1) * 128],
                                      tp[:, :])

            def attn_block(hp, j):
                fb = hp * T
                ni = 4 * j + 4
                # yps slot (q4, half) = 2*q4+half: slots 0-3 in ya, 4-7 in yb;
                # 66 cols each (65 used: col 64 = softmax denominator).
                ytiles = [
                    pool_yps.tile([128, 264], f32, tag="yps", name=f"y{hp}_{j}_{m}")
                    for m in range(2)
                ]
                ets = [None] * ni

                def emit_score(i):
                    import contextlib
                    hoist = (tc.high_priority(None if KPRI == 0 else KPRI)
                             if KPRI >= 0 else contextlib.nullcontext())
                    with hoist:
                        _emit_score(i)

                def _emit_score(i):
                    d0 = max(128 * (i - 4 * j), 0)
                    sc = pool_sc.tile([128, 1024], f32, tag="sc",
                                      name=f"sc{hp}_{j}_{i}")
                    for half in range(2):
                        po = 64 * half
                        nc.tensor.matmul(
                            sc[:, half * 512 + d0:(half + 1) * 512],
                            kT[po:po + 64, fb + i * 128: fb + (i + 1) * 128],
                            qT[po:po + 64, fb + j * 512 + d0: fb + (j + 1) * 512],
                            start=True, stop=True,
                        )
                    et = epool.tile([128, 1024], bf16, tag="exp",
                                    name=f"et{hp}_{j}_{i}")
                    et2 = et[:, :].rearrange("p (g q) -> p g q", g=2)
                    sc2 = sc[:, :].rearrange("p (g q) -> p g q", g=2)
                    nc.scalar.activation(
                        et2[:, :, d0:512], sc2[:, :, d0:512], Exp,
                        scale=float(SCALE / (WS * WS)),
                    )
                    if i >= 4 * j:  # diagonal chunk: causal mask (post-exp)
                        teng = nc.gpsimd if KTRI == "pool" else nc.vector
                        for half in range(2):
                            sl = slice(half * 512 + d0, half * 512 + d0 + 128)
                            teng.tensor_mul(et[:, sl], et[:, sl], tri[:, :])
                    ets[i] = et

                def emit_av(i):
                    import contextlib
                    delay = (tc.high_priority(-KPRA) if KPRA > 0
                             else contextlib.nullcontext())
                    with delay:
                        _emit_av(i)

                def _emit_av(i):
                    # PSUM start_tensor_calc marks the whole 2KB bank pending-
                    # zero, so: ONE start per yps bank (its first matmul); the
                    # other slots' first writes land on pending-zero bytes and
                    # overwrite; ONE stop on the bank's last matmul. q4 runs
                    # descending so the tri-masked diagonal chunk (q4 = i-4j)
                    # is consumed last, hiding the Pool mask latency.
                    et = ets[i]
                    for half in range(2):
                        h = 2 * hp + half
                        for q4 in range(3, -1, -1):
                            if 4 * j + q4 < i:
                                continue
                            s = 2 * q4 + half
                            yt = ytiles[s // 4]
                            off = (s % 4) * 66
                            bank_start = (i == 0 and half == 0
                                          and q4 % 2 == 1)
                            bank_stop = (half == 1 and q4 % 2 == 1
                                         and i == 4 * j + q4)
                            nc.tensor.matmul(
                                yt[:, off:off + 65],
                                et[:, half * 512 + q4 * 128: half * 512 + (q4 + 1) * 128],
                                Vv[:, h, i, :],
                                start=bank_start, stop=bank_stop,
                                skip_group_check=True,
                            )

                # yield one closure bundle per score tile; the global driver
                # interleaves score slots across block boundaries so the Act
                # exp stream never drains at a boundary.
                def make_av(i):
                    def run():
                        if hp == 0 and i >= 4 * j:
                            # V tile i filler must be emitted before av(i)
                            while n_popped[0] < vpos.get(i, 0):
                                pop_filler()
                        emit_av(i)
                        qa = i - 4 * j
                        if qa >= 0:
                            yp = emit_complA(hp, j, qa, ytiles)
                            bq_.append((slot[0] + KBLAG,
                                        lambda: emit_complB_T(hp, 4 * j + qa, yp)))
                            if hp == 1:
                                tail = (j == NJ - 1)
                                pq_.append((slot[0] + KBLAG + 1,
                                            lambda: emit_proj(4 * j + qa, tail)))
                    return run

                pace = KPACE if KPACE != 9 else (1 if j >= 2 else 3)
                for i in range(ni):
                    yield emit_score, i, make_av(i), pace

            if KORDER == "pair":
                order = [(hp, j) for j in range(NJ) for hp in range(2)]
            elif KORDER == "hprev":
                order = ([(0, j) for j in range(NJ)]
                         + [(1, j) for j in range(NJ - 1, -1, -1)])
            else:
                order = [(hp, j) for hp in range(2) for j in range(NJ)]
            req_so_far = 0
            reqs = []
            for hp, j in order:
                req_so_far = max(req_so_far, cum.get((hp, j), 0))
                reqs.append(req_so_far)

            # ---- global slot driver: one score per slot, continuous ----
            bq_ = []        # pending complB (transpose+copy), 1 per slot
            pq_ = []        # pending proj, 1 per slot
            avq_ = []       # pending av closures, run KLAG slots late
            slot = [0]

            def run_slot(score_fn=None, si=None, av_fn=None, pace=None):
                if score_fn is not None:
                    score_fn(si)
                if av_fn is not None:
                    avq_.append(av_fn)
                if bq_ and (bq_[0][0] <= slot[0] or score_fn is None):
                    bq_.pop(0)[1]()
                if slot[0] % (pace or 2) == 0:
                    pop_filler()
                if pq_ and (pq_[0][0] <= slot[0] or score_fn is None):
                    pq_.pop(0)[1]()
                while len(avq_) > (KLAG - 1 if score_fn is not None else 0):
                    avq_.pop(0)()
                slot[0] += 1

            for (hp, j), req in zip(order, reqs):
                while n_popped[0] < req:
                    pop_filler()
                for score_fn, si, av_fn, pace in attn_block(hp, j):
                    run_slot(score_fn, si, av_fn, pace)
            # flush: trailing avs, completions, projections, leftover fillers
            for _ in range(2 + KLAG):
                run_slot()
            while fillers:
                pop_filler()

            if DBG:
                nc.sync.dma_start(out=qT_dump[:, :], in_=qT[:, :])
                nc.sync.dma_start(out=kT_dump[:, :], in_=kT[:, :])
                nc.sync.dma_start(out=yT_dump[:, :], in_=yT[:, :])
                nc.sync.dma_start(out=V_dump[:, :], in_=V[:, :])

    nc.compile()
    return nc


def get_program():
    if "nc" not in _CACHE:
        _CACHE["nc"] = _build_program()
    return _CACHE["nc"]


def _pack_cmajor(a):
    """[C_rows, N] -> [128, (C_rows/128)*N] with chunk c at [:, c*N:(c+1)*N]."""
    rows, n = a.shape
    return np.ascontiguousarray(
        a.reshape(rows // 128, 128, n).transpose(1, 0, 2).reshape(128, -1))


def _pack_banded(a):
    """x[b].T [1024, 2048] -> [128, 16384], col = ts*4096 + c*512 + t'."""
    return np.ascontiguousarray(
        a.reshape(8, 128, 4, 512).transpose(1, 2, 0, 3).reshape(128, 16384))


def _split_fp8(a):
    """a (f32) -> (hi, lo) fp8e4m3 with hi + lo ~= a (error ~2^-8 rel)."""
    hi = a.astype(F8)
    lo = (a - hi.astype(np.float32)).astype(F8)
    return hi, lo


def make_in_maps(x, W_attn, b_attn, W_proj):
    """Host-side sharding: per-core input dict."""
    x = np.asarray(x, np.float32)
    W_attn = np.asarray(W_attn, np.float32)
    b_attn = np.asarray(b_attn, np.float32)
    W_proj = np.asarray(W_proj, np.float32)

    tk = np.arange(128)[:, None]
    tq = np.arange(128)[None, :]
    tri = (tq >= tk).astype(BF16)
    eye = np.eye(128, dtype=BF16)
    mbf = np.ascontiguousarray(np.concatenate([tri, eye], axis=1))

    x_b = []
    for b in range(B):
        hi, lo = _split_fp8(x[b].T)
        hi, lo = _pack_banded(hi), _pack_banded(lo)
        # interleave bands: [ts*8192 + h*4096 + ...]
        xi = np.empty((128, 32768), dtype=F8)
        for ts in range(4):
            xi[:, ts * 8192:ts * 8192 + 4096] = hi[:, ts * 4096:(ts + 1) * 4096]
            xi[:, ts * 8192 + 4096:(ts + 1) * 8192] = lo[:, ts * 4096:(ts + 1) * 4096]
        x_b.append(np.ascontiguousarray(xi))

    in_maps = []
    for g in range(N_CORES):
        b, hg = divmod(g, 4)
        cs = slice(hg * DL, (hg + 1) * DL)
        wqh, wql = _split_fp8(WS * W_attn[:, 0 * C:1 * C][:, cs])
        wkh, wkl = _split_fp8(WS * W_attn[:, 1 * C:2 * C][:, cs])
        wvh, wvl = _split_fp8(WS * W_attn[:, 2 * C:3 * C][:, cs])
        # per-dc packs [kh | kl | qh | ql], each part [128, 1024] c-major
        def _wpack(dc):
            sl = slice(dc * 128, (dc + 1) * 128)
            return np.ascontiguousarray(np.concatenate(
                [_pack_cmajor(wkh[:, sl]), _pack_cmajor(wkl[:, sl]),
                 _pack_cmajor(wqh[:, sl]), _pack_cmajor(wql[:, sl])], axis=1))
        wp = _pack_cmajor(W_proj[cs, :].astype(BF16))
        bq = np.ascontiguousarray(WS * b_attn[0 * C:1 * C][cs].reshape(2, 128).T)
        bk = np.ascontiguousarray(WS * b_attn[1 * C:2 * C][cs].reshape(2, 128).T)
        bvr = np.tile(WS * b_attn[2 * C:3 * C][cs][None, :], (128, 1))
        m32 = np.ascontiguousarray(
            np.concatenate([bq, bk, bvr], axis=1).astype(np.float32))
        in_maps.append({
            "xp": x_b[b],
            "w0p": _wpack(0), "w1p": _wpack(1),
            "wvp": np.ascontiguousarray(np.concatenate(
                [_pack_cmajor(wvh), _pack_cmajor(wvl)], axis=1)),
            "wpp": wp, "m32": m32, "mbf": mbf,
        })
    return in_maps


def assemble_output(results, b_proj):
    """results: per-core dicts with 'out' [T, C] fp16 partials."""
    b_proj = np.asarray(b_proj, np.float32)
    out = np.zeros((B, T, C), np.float32)
    for g in range(N_CORES):
        out[g // 4] += np.asarray(results[g]["out"], np.float32)
    out += b_proj[None, None, :]
    return out


def kernel(x, W_attn, b_attn, W_proj, b_proj):
    from concourse.bass_utils import run_bass_kernel_spmd

    nc = get_program()
    in_maps = make_in_maps(x, W_attn, b_attn, W_proj)
    res = run_bass_kernel_spmd(nc, in_maps, list(range(N_CORES)))
    return assemble_output(res.results, b_proj)
